# revision 19
# baseline (speedup 1.0000x reference)
"""Trainium2 Bass kernel for nn_DKTAccum_no_tempo_Model (DKT with count-feature LSTM).

Strategy (8 NeuronCores, pure data parallel over batch, 16 rows/core):
  Phase A: stream x (fp16, channel-major, de-interleaved), embed = x @ Wx on
           PE, interaction counts via DVE tensor_tensor_scan, count gather via
           pair-indicator multiply + ones-matmul, log1p on ACT, DMA-expand
           into the segmented LSTM layout.
  Phase B: LSTM in a flipped layout: partitions = 128 (batch,segment)
           columns per pipe, free dim = 400 gate outputs. Each round-pipe is
           2 matmuls (weights = EMB / state slices, moving = packed LSTM
           kernels), one [*,300] sigmoid (gates i,f,o), two [*,100] tanh,
           4 DVE elementwise ops, and a PE transpose of h back into the
           [hidden, column] state layout. 16 segments x 32 steps + 32-step
           warmup = 64 lockstep rounds; 2 pipes interleave.
  Phase C: z = h @ Wo + bo on PE (4 segments per matmul), dot with one-hot q
           via DVE accumulate, single sigmoid on the reduced [128,64] tile.
"""
import sys

sys.path.insert(0, "/opt/trn_rl_repo")

import numpy as np

import concourse.bass as bass
import concourse.tile as tile
from concourse import bacc, mybir
from concourse.bass_utils import run_bass_kernel_spmd

# ---- problem constants -----------------------------------------------------
B, T, S = 128, 500, 200          # batch, seq, skills
E, H = 100, 100                  # embed dim, lstm hidden
NCORES = 8
BC = B // NCORES                 # 16 batch rows per core
G = 16                           # time segments per row
SEG = 32                         # real steps per segment (16*32 = 512 >= 500)
W = 32                           # warmup steps per segment
ROUNDS = W + SEG                 # 64 lockstep rounds
TP = G * SEG                     # padded T = 512
U = BC * G                       # 256 (batch, segment) columns per core
ZBLK = ROUNDS + 1                # 65 state cols per block
EBLK = ROUNDS                    # 64 embed cols per block
NZC = U * ZBLK                   # 16640
NEC = U * EBLK                   # 16384
SGB = W + TP                     # 544 staging cols per (b, feat)
F16 = mybir.dt.float16
F32 = mybir.dt.float32
AOP = mybir.AluOpType
AF = mybir.ActivationFunctionType

_cache = {}


def _build():
    nc = bacc.Bacc(
        "TRN2",
        target_bir_lowering=False,
        debug=False,
        enable_asserts=False,
        num_devices=NCORES,
    )
    xd_d = nc.dram_tensor("xd", [BC, E, 4 * TP], F16, kind="ExternalInput")
    qh_d = nc.dram_tensor("qh", [BC, 128, 4 * S], F16, kind="ExternalInput")
    wxe_d = nc.dram_tensor("wxe", [4, E, 128], F16, kind="ExternalInput")
    ke_d = nc.dram_tensor("ke", [E, 4 * H], F16, kind="ExternalInput")
    rk_d = nc.dram_tensor("rk", [104, 4 * H], F16, kind="ExternalInput")
    wob_d = nc.dram_tensor("wob", [104, S], F16, kind="ExternalInput")
    ones_d = nc.dram_tensor("onesrow", [1, NZC], F16, kind="ExternalInput")
    ident_d = nc.dram_tensor("ident", [128, 128], F16, kind="ExternalInput")
    yout_d = nc.dram_tensor("yout", [128, BC * 4], F32, kind="ExternalOutput")

    with tile.TileContext(nc) as tc:
        _emit(tc, nc, xd_d, qh_d, wxe_d, ke_d, rk_d, wob_d, ones_d, ident_d,
              yout_d)
    nc.compile()
    return nc


def _emit(tc, nc, xd_d, qh_d, wxe_d, ke_d, rk_d, wob_d, ones_d, ident_d,
          yout_d):
    from contextlib import ExitStack

    with ExitStack() as ctx:
        big = ctx.enter_context(tc.tile_pool(name="big", bufs=1))
        wpool = ctx.enter_context(tc.tile_pool(name="w", bufs=1))
        xdp = ctx.enter_context(tc.tile_pool(name="xd", bufs=2))
        ctp = ctx.enter_context(tc.tile_pool(name="ct", bufs=2))
        s2p = ctx.enter_context(tc.tile_pool(name="s2", bufs=2))
        ep = ctx.enter_context(tc.tile_pool(name="emul", bufs=2))
        gp = ctx.enter_context(tc.tile_pool(name="gates", bufs=2))
        cp = ctx.enter_context(tc.tile_pool(name="cstate", bufs=2))
        clp = ctx.enter_context(tc.tile_pool(name="clog", bufs=2))

        # ---- persistent tensors -------------------------------------------
        ZR = big.tile([128, NZC], F16)    # h(0:100), feats(100:103), ones(103)
        EMB = big.tile([128, NEC], F16)   # embed rows 0:100, blocked
        QT = big.tile([128, BC * 4 * S], F16)
        STG = big.tile([1, BC * 3 * SGB], F16)  # log-count staging row
        ZERO = big.tile([E, TP], F16)
        Y = big.tile([128, BC * 4], F32)

        WXE = [wpool.tile([E, 128], F16, tag=f"wxe{k}", name=f"WXE{k}")
               for k in range(4)]
        KEA = wpool.tile([E, 4 * H], F16, tag="kea")
        RKA = wpool.tile([104, 4 * H], F16, tag="rka")
        WOB = wpool.tile([104, S], F16, tag="wob")
        ONES1 = wpool.tile([E, 1], F16, tag="ones1")
        IDN = wpool.tile([128, 128], F16, tag="idn")

        for k in range(4):
            nc.sync.dma_start(WXE[k][:], wxe_d.ap()[k])
        nc.sync.dma_start(KEA[:], ke_d.ap()[:])
        nc.sync.dma_start(RKA[:], rk_d.ap()[:])
        nc.sync.dma_start(WOB[:], wob_d.ap()[:])
        nc.sync.dma_start(IDN[:], ident_d.ap()[:])
        nc.sync.dma_start(ZR[103:104, :], ones_d.ap()[:])

        # targeted zero-fills on the otherwise idle Pool engine (ZERO first:
        # the scans wait on it and the Pool queue is in-order)
        zru = ZR[:].rearrange("p (u q) -> p u q", u=U, q=ZBLK)
        embv = EMB[:].rearrange("p (b j q) -> p b j q", b=BC, j=G, q=EBLK)
        nc.gpsimd.memset(ZERO[:], 0.0)
        nc.gpsimd.memset(ONES1[:], 1.0)
        s0 = STG[0:1, :]
        nc.gpsimd.memset(                                # tau<0 guard cols
            bass.AP(s0.tensor, s0.offset, [s0.ap[0], [SGB, 3 * BC], [1, W]]),
            0.0)
        nc.gpsimd.memset(zru[0:E, :, 0:1], 0.0)          # h init col
        nc.vector.memset(zru[96:103, :, ROUNDS:ZBLK], 0.0)
        nc.gpsimd.memset(embv[0:E, :, 0, 0:W], 0.0)      # block-0 warmup
        nc.gpsimd.memset(Y[:], 0.0)

        # ---- phase A -------------------------------------------------------
        ctxA = ExitStack()
        pse = ctxA.enter_context(tc.tile_pool(name="pse", bufs=2, space="PSUM"))
        psg = ctxA.enter_context(tc.tile_pool(name="psg", bufs=2, space="PSUM"))

        for b in range(BC):
            XT = xdp.tile([E, 4 * TP], F16, tag="xd", name=f"xt{b}")
            nc.sync.dma_start(XT[:], xd_d.ap()[b])
            xt = [XT[:, TP * k:TP * (k + 1)] for k in range(4)]

            # embed: psum_e = sum_k WXE[k].T @ x[k]   -> [128(E pad), TP]
            pe = pse.tile([128, TP], F32, tag="pe", name=f"pe{b}")
            for k in range(4):
                nc.tensor.matmul(pe[:], WXE[k][:], xt[k], start=(k == 0),
                                 stop=(k == 3), skip_group_check=True)
            # scatter embed into EMB block windows (fp16)
            nc.scalar.copy(embv[0:E, b, 0, W:EBLK], pe[0:E, 0:SEG])
            pes = pe[0:E, 0:TP]
            src3 = bass.AP(pes.tensor, pes.offset,
                           [pes.ap[0], [SEG, G - 1], [1, EBLK]])
            nc.scalar.copy(embv[0:E, b, 1:G, 0:EBLK], src3)

            # inclusive count cumsum over t per channel chunk
            CT = ctp.tile([E, 4 * TP], F16, tag="ct", name=f"ct{b}")
            for k in range(4):
                nc.vector.tensor_tensor_scan(
                    CT[:, TP * k:TP * (k + 1)], xt[k], ZERO[:], 0.0,
                    op0=AOP.add, op1=AOP.add)

            # pair indicator s2 = x_corr + x_incorr (both skill chunks at once)
            s2 = s2p.tile([E, 2 * TP], F16, tag="s2", name=f"s2_{b}")
            nc.vector.tensor_tensor(s2[:], XT[:, 0:2 * TP], XT[:, 2 * TP:],
                                    op=AOP.add)
            emc = ep.tile([E, 2 * TP], F16, tag="emc", name=f"emc{b}")
            emi = ep.tile([E, 2 * TP], F16, tag="emi", name=f"emi{b}")
            nc.vector.tensor_tensor(emc[:], CT[:, 0:2 * TP], s2[:],
                                    op=AOP.mult)
            nc.gpsimd.tensor_tensor(emi[:], CT[:, 2 * TP:], s2[:],
                                    op=AOP.mult)
            pcc = psg.tile([1, TP], F32, tag="pcc", name=f"pcc{b}")
            pic = psg.tile([1, TP], F32, tag="pic", name=f"pic{b}")
            for k in range(2):
                nc.tensor.matmul(pcc[:], ONES1[:],
                                 emc[:, TP * k:TP * (k + 1)],
                                 start=(k == 0), stop=(k == 1),
                                 skip_group_check=True)
            for k in range(2):
                nc.tensor.matmul(pic[:], ONES1[:],
                                 emi[:, TP * k:TP * (k + 1)],
                                 start=(k == 0), stop=(k == 1),
                                 skip_group_check=True)

            # log1p -> staging row; ss = cc + ic
            sv = STG[0:1, 3 * SGB * b:3 * SGB * (b + 1)]
            nc.scalar.activation(sv[:, W:SGB], pcc[:], AF.Ln,
                                 bias=1.0, scale=1.0)
            nc.scalar.activation(sv[:, SGB + W:2 * SGB], pic[:], AF.Ln,
                                 bias=1.0, scale=1.0)
            nc.vector.tensor_tensor(sv[:, 2 * SGB + W:3 * SGB],
                                    sv[:, W:SGB], sv[:, SGB + W:2 * SGB],
                                    op=AOP.add)

            # expand staging into ZR feat rows: overlapping strided reads do
            # the per-block window expansion; the W guard columns supply
            # zeros for tau < 0.
            for f in range(3):
                s3 = STG[0:1, 3 * SGB * b + SGB * f:]
                src = bass.AP(s3.tensor, s3.offset,
                              [s3.ap[0], [SEG, G], [1, ROUNDS]])
                z1 = ZR[100 + f:101 + f, (b * G) * ZBLK:]
                dst = bass.AP(z1.tensor, z1.offset,
                              [z1.ap[0], [ZBLK, G], [1, ROUNDS]])
                eng = nc.sync if f != 1 else nc.scalar
                eng.dma_start(dst, src)

        # q prefetch (needed in phase C only)
        for b in range(BC):
            nc.scalar.dma_start(QT[:, 4 * S * b:4 * S * (b + 1)],
                                qh_d.ap()[b])

        ctxA.close()

        # ---- phase B: lockstep segmented LSTM, flipped layout -------------
        ctxB = ExitStack()
        psz = ctxB.enter_context(tc.tile_pool(name="psz", bufs=2, space="PSUM"))
        pst = ctxB.enter_context(tc.tile_pool(name="pst", bufs=2, space="PSUM"))
        embu = EMB[:].rearrange("p (u q) -> p u q", u=U, q=EBLK)

        c_prev = []
        for p_ in range(2):
            c0 = cp.tile([128, H], F32, tag=f"c{p_}", name=f"c_init{p_}")
            nc.gpsimd.memset(c0[:], 0.0)
            c_prev.append(c0)

        st = [{}, {}]   # per-pipe in-flight tiles

        def s1(p_, r):
            u0 = 128 * p_
            pz = psz.tile([128, 512], F32, tag=f"pz{p_}", name=f"pz{p_}_{r}")
            nc.tensor.matmul(pz[:, 0:4 * H], embu[0:E, u0:u0 + 128, r],
                             KEA[:], start=True, stop=False,
                             skip_group_check=True)
            nc.tensor.matmul(pz[:, 0:4 * H], zru[0:104, u0:u0 + 128, r],
                             RKA[:], start=False, stop=True,
                             skip_group_check=True)
            sig = gp.tile([128, 3 * H], F16, tag=f"sig{p_}",
                          name=f"sig{p_}_{r}")
            tg = gp.tile([128, H], F32, tag=f"tg{p_}", name=f"tg{p_}_{r}")
            nc.scalar.activation(sig[:], pz[:, 0:3 * H], AF.Sigmoid)
            nc.scalar.activation(tg[:], pz[:, 3 * H:4 * H], AF.Tanh)
            st[p_]["sig"], st[p_]["tg"] = sig, tg

        def s2(p_, r):
            sig, tg = st[p_]["sig"], st[p_]["tg"]
            u_t = gp.tile([128, H], F32, tag=f"u{p_}", name=f"u{p_}_{r}")
            v_t = gp.tile([128, H], F32, tag=f"v{p_}", name=f"v{p_}_{r}")
            nc.vector.tensor_tensor(u_t[:], sig[:, H:2 * H], c_prev[p_][:],
                                    op=AOP.mult)
            nc.vector.tensor_tensor(v_t[:], sig[:, 0:H], tg[:], op=AOP.mult)
            c_new = cp.tile([128, H], F32, tag=f"c{p_}", name=f"cn{p_}_{r}")
            nc.vector.tensor_tensor(c_new[:], u_t[:], v_t[:], op=AOP.add)
            tc_t = gp.tile([128, H], F32, tag=f"tc{p_}", name=f"tc{p_}_{r}")
            nc.scalar.activation(tc_t[:], c_new[:], AF.Tanh)
            c_prev[p_] = c_new
            st[p_]["tc"] = tc_t

        def s3(p_, r):
            u0 = 128 * p_
            hh = gp.tile([128, H], F16, tag=f"h{p_}", name=f"h{p_}_{r}")
            nc.vector.tensor_tensor(hh[:], st[p_]["sig"][:, 2 * H:3 * H],
                                    st[p_]["tc"][:], op=AOP.mult)
            pt = pst.tile([128, 128], F16, tag=f"pt{p_}", name=f"pt{p_}_{r}")
            nc.tensor.transpose(pt[0:H, :], hh[:], IDN[:])
            nc.vector.tensor_copy(zru[0:H, u0:u0 + 128, r + 1], pt[0:H, :])

        # software-pipelined: pipe 1 runs half a round behind pipe 0 so each
        # engine alternates between the two dependency chains
        s1(0, 0)
        for r in range(ROUNDS):
            s2(0, r)
            s1(1, r)
            s3(0, r)
            s2(1, r)
            if r + 1 < ROUNDS:
                s1(0, r + 1)
            s3(1, r)

        ctxB.close()

        # ---- phase C: output layer ----------------------------------------
        ctxC = ExitStack()
        psc = ctxC.enter_context(tc.tile_pool(name="psc", bufs=3, space="PSUM"))
        zrv = ZR[:].rearrange("p (b j q) -> p b j q", b=BC, j=G, q=ZBLK)
        HC = big.tile([104, BC * G * SEG], F16)

        # gather the valid h windows into a contiguous tile so the output
        # matmuls get 2D weight access patterns (SBUF->SBUF DMA handles the
        # 3D strided read)
        for b in range(BC):
            s4 = zrv[0:104, b, 0, W + 1:]
            src = bass.AP(s4.tensor, s4.offset,
                          [s4.ap[0], [ZBLK, G], [1, SEG]])
            eng = nc.sync if b % 2 == 0 else nc.scalar
            eng.dma_start(HC[:, G * SEG * b:G * SEG * (b + 1)], src)

        for b in range(BC):
            for jg in range(4):
                zp = psc.tile([128, S], F32, tag="zp", name=f"zp{b}_{jg}")
                nc.tensor.matmul(zp[:],
                                 HC[:, G * SEG * b + 128 * jg:
                                    G * SEG * b + 128 * (jg + 1)],
                                 WOB[:], start=True, stop=True,
                                 skip_group_check=True)
                sl = clp.tile([128, S], F16, tag="sl", name=f"sl{b}_{jg}")
                nc.scalar.activation(sl[:], zp[:], AF.Sigmoid)
                scr = clp.tile([128, S], F16, tag="scr", name=f"scr{b}_{jg}")
                nc.vector.scalar_tensor_tensor(
                    scr[:], sl[:], 1.0,
                    QT[:, 4 * S * b + S * jg:4 * S * b + S * (jg + 1)],
                    op0=AOP.mult, op1=AOP.mult,
                    accum_out=Y[:, 4 * b + jg:4 * b + jg + 1])

        nc.sync.dma_start(yout_d.ap()[:], Y[:])
        ctxC.close()


# ---- host side -------------------------------------------------------------
def _prep(inputs):
    x = np.asarray(inputs["x"], np.float32)
    q = np.asarray(inputs["q"], np.float32)
    Wx = np.asarray(inputs["Wx"], np.float32)
    bx = np.asarray(inputs["bx"], np.float32)
    lstm_k = np.asarray(inputs["lstm_k"], np.float32)
    lstm_rk = np.asarray(inputs["lstm_rk"], np.float32)
    lstm_b = np.asarray(inputs["lstm_b"], np.float32)
    Wo = np.asarray(inputs["Wo"], np.float32)
    bo = np.asarray(inputs["bo"], np.float32)

    # channel de-interleave: deint[..., skill + 200*bit] = orig[..., 2*skill+bit]
    perm = np.empty(2 * S, np.int64)
    sk = np.arange(S)
    perm[sk] = 2 * sk
    perm[S + sk] = 2 * sk + 1

    xd = x[:, :, perm].transpose(0, 2, 1)                 # [B, 400, T]
    xdp = np.zeros((B, 4, E, TP), np.float16)
    xdp[:, :, :, :T] = xd.reshape(B, 4, E, T).astype(np.float16)
    xdp = xdp.transpose(0, 2, 1, 3).reshape(B, E, 4 * TP)

    # q blocked: part p = 32*seg_in_group + step, col = jg*S + s
    qtmp = np.zeros((B, TP, S), np.float16)
    qtmp[:, :T, :] = q.astype(np.float16)
    qhp = np.ascontiguousarray(
        qtmp.reshape(B, 4, 128, S).transpose(0, 2, 1, 3).reshape(
            B, 128, 4 * S))

    # gate reorder [i,f,g,o] -> [i,f,o,g]
    gperm = np.concatenate([np.arange(H), H + np.arange(H),
                            3 * H + np.arange(H), 2 * H + np.arange(H)])
    k_r = lstm_k[:, gperm]
    rk_r = lstm_rk[:, gperm]
    b_r = lstm_b[gperm]
    Wxd = Wx[perm]

    bias_row = bx @ k_r[:E] + b_r

    wxe = np.zeros((4, E, 128), np.float16)
    wxe[:, :, :E] = Wxd.reshape(4, E, E).astype(np.float16)

    kea = k_r[:E].astype(np.float16)                      # [E, 400]
    rka = np.zeros((104, 4 * H), np.float16)
    rka[0:H] = rk_r.astype(np.float16)
    rka[100:103] = k_r[E:E + 3].astype(np.float16)
    rka[103] = bias_row.astype(np.float16)

    wob = np.zeros((104, S), np.float16)
    wob[0:H] = Wo.astype(np.float16)
    wob[103] = bo.astype(np.float16)

    onesrow = np.zeros((1, NZC), np.float16)
    qq = np.arange(ZBLK)
    for j in range(G):
        tau = SEG * j - W + qq
        valid = (tau >= 0) & (tau < T)
        for b in range(BC):
            base = (b * G + j) * ZBLK
            onesrow[0, base:base + ZBLK][valid] = 1.0

    ident = np.eye(128, dtype=np.float16)
    return xdp, qhp, wxe, kea, rka, wob, onesrow, ident


def kernel(**inputs):
    if "nc" not in _cache:
        _cache["nc"] = _build()
    nc = _cache["nc"]

    xdp, qhp, wxe, kea, rka, wob, onesrow, ident = _prep(inputs)

    in_maps = []
    for c in range(NCORES):
        sl = slice(c * BC, (c + 1) * BC)
        in_maps.append({
            "xd": np.ascontiguousarray(xdp[sl]),
            "qh": np.ascontiguousarray(qhp[sl]),
            "wxe": wxe, "ke": kea, "rk": rka, "wob": wob,
            "onesrow": onesrow, "ident": ident,
        })

    res = run_bass_kernel_spmd(nc, in_maps, core_ids=list(range(NCORES)))

    y = np.zeros((B, T, 1), np.float32)
    for c in range(NCORES):
        yo = np.asarray(res.results[c]["yout"])     # [128, BC*4]
        yc = yo.reshape(128, BC, 4).transpose(1, 2, 0).reshape(BC, 4 * 128)
        y[c * BC:(c + 1) * BC, :, 0] = yc[:, :T]
    return y


# revision 26
# speedup vs baseline: 1.1177x; 1.1177x over previous
"""Trainium2 Bass kernel for nn_DKTAccum_no_tempo_Model (DKT with count-feature LSTM).

Strategy (8 NeuronCores, pure data parallel over batch, 16 rows/core):
  Phase A: stream x (fp16, channel-major, de-interleaved), embed = x @ Wx on
           PE, interaction counts via DVE tensor_tensor_scan, count gather via
           pair-indicator multiply + ones-matmul, log1p on ACT, DMA-expand
           into the segmented LSTM layout.
  Phase B: LSTM in a flipped layout: partitions = 128 (batch,segment)
           columns per pipe, free dim = 400 gate outputs. Each round-pipe is
           2 matmuls (weights = EMB / state slices, moving = packed LSTM
           kernels), ONE [*,400] sigmoid (g-gate weights pre-scaled x2 so
           tanh(g) = 2*sig(2g)-1), one tanh(c), 5 DVE elementwise ops, and a
           PE transpose of h back into the [hidden, column] state layout.
           24 segments x 21 steps + 30-step warmup = 51 lockstep rounds;
           3 pipes stagger through the recurrence chain.
  Phase C: gather h into time-order (SBUF DMA), z = h @ Wo on PE, dot with
           one-hot q via DVE/Pool accumulate, sigmoid on the reduced tile.
"""
import sys

sys.path.insert(0, "/opt/trn_rl_repo")

import numpy as np

import concourse.bass as bass
import concourse.tile as tile
from concourse import bacc, mybir
from concourse.bass_utils import run_bass_kernel_spmd

# ---- problem constants -----------------------------------------------------
B, T, S = 128, 500, 200          # batch, seq, skills
E, H = 100, 100                  # embed dim, lstm hidden
NCORES = 8
BC = B // NCORES                 # 16 batch rows per core
G = 24                           # time segments per row
SEG = 21                         # real steps per segment (24*21 = 504 >= 500)
W = 30                           # warmup steps per segment
ROUNDS = W + SEG                 # 51 lockstep rounds
TP = G * SEG                     # padded T = 504
U = BC * G                       # 384 (batch, segment) columns per core
NP = 3                           # pipes (128 columns each)
ZBLK = ROUNDS + 1                # 52 state cols per block
EBLK = ROUNDS                    # 51 embed cols per block
NZC = U * ZBLK                   # 19968
NEC = U * EBLK                   # 19584
SGB = W + TP                     # 534 staging cols per (b, feat)
F16 = mybir.dt.float16
F32 = mybir.dt.float32
AOP = mybir.AluOpType
AF = mybir.ActivationFunctionType

_cache = {}


def _build():
    nc = bacc.Bacc(
        "TRN2",
        target_bir_lowering=False,
        debug=False,
        enable_asserts=False,
        num_devices=NCORES,
    )
    xd_d = nc.dram_tensor("xd", [BC, E, 4 * TP], F16, kind="ExternalInput")
    qh_d = nc.dram_tensor("qh", [BC, 128, 4 * S], F16, kind="ExternalInput")
    wxe_d = nc.dram_tensor("wxe", [4, E, 128], F16, kind="ExternalInput")
    ke_d = nc.dram_tensor("ke", [E, 4 * H], F16, kind="ExternalInput")
    rk_d = nc.dram_tensor("rk", [104, 4 * H], F16, kind="ExternalInput")
    wob_d = nc.dram_tensor("wob", [104, S], F16, kind="ExternalInput")
    ones_d = nc.dram_tensor("onesrow", [1, NZC], F16, kind="ExternalInput")
    ident_d = nc.dram_tensor("ident", [128, 128], F16, kind="ExternalInput")
    yout_d = nc.dram_tensor("yout", [128, BC * 4], F32, kind="ExternalOutput")

    with tile.TileContext(nc) as tc:
        _emit(tc, nc, xd_d, qh_d, wxe_d, ke_d, rk_d, wob_d, ones_d, ident_d,
              yout_d)
    nc.compile()
    return nc


def _emit(tc, nc, xd_d, qh_d, wxe_d, ke_d, rk_d, wob_d, ones_d, ident_d,
          yout_d):
    from contextlib import ExitStack

    with ExitStack() as ctx:
        big = ctx.enter_context(tc.tile_pool(name="big", bufs=1))
        wpool = ctx.enter_context(tc.tile_pool(name="w", bufs=1))
        xdp = ctx.enter_context(tc.tile_pool(name="xd", bufs=3))
        ctp = ctx.enter_context(tc.tile_pool(name="ct", bufs=3))
        s2p = ctx.enter_context(tc.tile_pool(name="s2", bufs=3))
        ep = ctx.enter_context(tc.tile_pool(name="emul", bufs=3))
        gp = ctx.enter_context(tc.tile_pool(name="gates", bufs=2))
        cp = ctx.enter_context(tc.tile_pool(name="cstate", bufs=2))
        clp = ctx.enter_context(tc.tile_pool(name="clog", bufs=3))

        # ---- persistent tensors -------------------------------------------
        ZR = big.tile([128, NZC], F16)    # h(0:100), feats(100:103), ones(103)
        EMB = big.tile([128, NEC], F16)   # embed rows 0:100, blocked
        QT = big.tile([128, BC * 4 * S], F16)
        ZERO = big.tile([E, TP], F16)
        Y = big.tile([128, BC * 4], F32)
        stgp = ctx.enter_context(tc.tile_pool(name="stg", bufs=3))

        WXE = [wpool.tile([E, 128], F16, tag=f"wxe{k}", name=f"WXE{k}")
               for k in range(4)]
        KEA = wpool.tile([E, 4 * H], F16, tag="kea")
        RKA = wpool.tile([104, 4 * H], F16, tag="rka")
        WOB = wpool.tile([104, S], F16, tag="wob")
        ONES1 = wpool.tile([E, 1], F16, tag="ones1")
        IDN = wpool.tile([128, 128], F16, tag="idn")

        for k in range(4):
            nc.sync.dma_start(WXE[k][:], wxe_d.ap()[k])
        nc.sync.dma_start(KEA[:], ke_d.ap()[:])
        nc.sync.dma_start(RKA[:], rk_d.ap()[:])
        nc.sync.dma_start(WOB[:], wob_d.ap()[:])
        nc.sync.dma_start(IDN[:], ident_d.ap()[:])
        nc.sync.dma_start(ZR[103:104, :], ones_d.ap()[:])

        # targeted zero-fills on the otherwise idle Pool engine (ZERO first:
        # the scans wait on it and the Pool queue is in-order)
        zru = ZR[:].rearrange("p (u q) -> p u q", u=U, q=ZBLK)
        embv = EMB[:].rearrange("p (b j q) -> p b j q", b=BC, j=G, q=EBLK)
        nc.gpsimd.memset(ZERO[:], 0.0)
        nc.gpsimd.memset(ONES1[:], 1.0)
        nc.gpsimd.memset(zru[0:E, :, 0:1], 0.0)          # h init col
        nc.vector.memset(zru[96:103, :, ROUNDS:ZBLK], 0.0)
        nc.gpsimd.memset(embv[0:E, :, 0, 0:W], 0.0)      # block-0 warmup
        nc.gpsimd.memset(embv[0:E, :, 1, 0:W - SEG], 0.0)  # block-1 tau<0
        nc.gpsimd.memset(Y[:], 0.0)

        # ---- phase A -------------------------------------------------------
        ctxA = ExitStack()
        pse = ctxA.enter_context(tc.tile_pool(name="pse", bufs=2, space="PSUM"))
        psg = ctxA.enter_context(tc.tile_pool(name="psg", bufs=2, space="PSUM"))

        for b in range(BC):
            XT = xdp.tile([E, 4 * TP], F16, tag="xd", name=f"xt{b}")
            nc.sync.dma_start(XT[:], xd_d.ap()[b])
            xt = [XT[:, TP * k:TP * (k + 1)] for k in range(4)]

            # embed: psum_e = sum_k WXE[k].T @ x[k]   -> [128(E pad), TP]
            pe = pse.tile([128, TP], F32, tag="pe", name=f"pe{b}")
            for k in range(4):
                nc.tensor.matmul(pe[:], WXE[k][:], xt[k], start=(k == 0),
                                 stop=(k == 3), skip_group_check=True)
            # scatter embed into EMB block windows (fp16)
            nc.scalar.copy(embv[0:E, b, 0, W:EBLK], pe[0:E, 0:SEG])
            nc.scalar.copy(embv[0:E, b, 1, W - SEG:EBLK],
                           pe[0:E, 0:EBLK - W + SEG])
            pes = pe[0:E, 2 * SEG - W:]
            src3 = bass.AP(pes.tensor, pes.offset,
                           [pes.ap[0], [SEG, G - 2], [1, EBLK]])
            nc.scalar.copy(embv[0:E, b, 2:G, 0:EBLK], src3)

            # inclusive count cumsum over t per channel chunk
            CT = ctp.tile([E, 4 * TP], F16, tag="ct", name=f"ct{b}")
            for k in range(4):
                nc.vector.tensor_tensor_scan(
                    CT[:, TP * k:TP * (k + 1)], xt[k], ZERO[:], 0.0,
                    op0=AOP.add, op1=AOP.add)

            # pair indicator s2 = x_corr + x_incorr (both skill chunks at once)
            s2 = s2p.tile([E, 2 * TP], F16, tag="s2", name=f"s2_{b}")
            nc.vector.tensor_tensor(s2[:], XT[:, 0:2 * TP], XT[:, 2 * TP:],
                                    op=AOP.add)
            emc = ep.tile([E, 2 * TP], F16, tag="emc", name=f"emc{b}")
            emi = ep.tile([E, 2 * TP], F16, tag="emi", name=f"emi{b}")
            nc.vector.tensor_tensor(emc[:], CT[:, 0:2 * TP], s2[:],
                                    op=AOP.mult)
            nc.vector.tensor_tensor(emi[:], CT[:, 2 * TP:], s2[:],
                                    op=AOP.mult)
            pcc = psg.tile([1, TP], F32, tag="pcc", name=f"pcc{b}")
            pic = psg.tile([1, TP], F32, tag="pic", name=f"pic{b}")
            for k in range(2):
                nc.tensor.matmul(pcc[:], ONES1[:],
                                 emc[:, TP * k:TP * (k + 1)],
                                 start=(k == 0), stop=(k == 1),
                                 skip_group_check=True)
            for k in range(2):
                nc.tensor.matmul(pic[:], ONES1[:],
                                 emi[:, TP * k:TP * (k + 1)],
                                 start=(k == 0), stop=(k == 1),
                                 skip_group_check=True)

            # log1p -> staging row; ss = cc + ic
            sv = stgp.tile([1, 3 * SGB], F16, tag="stg", name=f"stg{b}")
            nc.gpsimd.memset(                            # tau<0 guard cols
                bass.AP(sv.tensor, sv.offset, [sv.ap[0], [SGB, 3], [1, W]]),
                0.0)
            nc.scalar.activation(sv[:, W:SGB], pcc[:], AF.Ln,
                                 bias=1.0, scale=1.0)
            nc.scalar.activation(sv[:, SGB + W:2 * SGB], pic[:], AF.Ln,
                                 bias=1.0, scale=1.0)
            nc.vector.tensor_tensor(sv[:, 2 * SGB + W:3 * SGB],
                                    sv[:, W:SGB], sv[:, SGB + W:2 * SGB],
                                    op=AOP.add)

            # expand staging into ZR feat rows: overlapping strided reads do
            # the per-block window expansion; the W guard columns supply
            # zeros for tau < 0.
            for f in range(3):
                s3 = sv[0:1, SGB * f:]
                src = bass.AP(s3.tensor, s3.offset,
                              [s3.ap[0], [SEG, G], [1, ROUNDS]])
                z1 = ZR[100 + f:101 + f, (b * G) * ZBLK:]
                dst = bass.AP(z1.tensor, z1.offset,
                              [z1.ap[0], [ZBLK, G], [1, ROUNDS]])
                nc.sync.dma_start(dst, src)

        # q prefetch (needed in phase C only; sync queue drains in phase B)
        for b in range(BC):
            nc.sync.dma_start(QT[:, 4 * S * b:4 * S * (b + 1)], qh_d.ap()[b])

        ctxA.close()

        # ---- phase B: lockstep segmented LSTM, flipped layout -------------
        ctxB = ExitStack()
        psz = ctxB.enter_context(tc.tile_pool(name="psz", bufs=2, space="PSUM"))
        pst = ctxB.enter_context(tc.tile_pool(name="pst", bufs=2, space="PSUM"))
        embu = EMB[:].rearrange("p (u q) -> p u q", u=U, q=EBLK)

        c_prev = []
        for p_ in range(NP):
            c0 = cp.tile([128, H], F32, tag=f"c{p_}", name=f"c_init{p_}")
            nc.gpsimd.memset(c0[:], 0.0)
            c_prev.append(c0)

        st = [{} for _ in range(NP)]   # per-pipe in-flight tiles

        def s1(p_, r):
            u0 = 128 * p_
            pz = psz.tile([128, 512], F32, tag=f"pz{p_}", name=f"pz{p_}_{r}")
            nc.tensor.matmul(pz[:, 0:4 * H], embu[0:E, u0:u0 + 128, r],
                             KEA[:], start=True, stop=False,
                             skip_group_check=True)
            nc.tensor.matmul(pz[:, 0:4 * H], zru[0:104, u0:u0 + 128, r],
                             RKA[:], start=False, stop=True,
                             skip_group_check=True)
            # gate cols [i f o g']; g' pre-scaled so sg = sig(2*g)
            sig = gp.tile([128, 4 * H], F16, tag=f"sig{p_}",
                          name=f"sig{p_}_{r}")
            nc.scalar.activation(sig[:], pz[:, 0:4 * H], AF.Sigmoid)
            st[p_]["sig"] = sig

        def s2(p_, r):
            sig = st[p_]["sig"]
            # c_new = f*c_prev - i + 2*(i*sg)   (== f*c + i*tanh(g))
            u_t = gp.tile([128, H], F32, tag=f"u{p_}", name=f"u{p_}_{r}")
            nc.vector.tensor_tensor(u_t[:], sig[:, H:2 * H], c_prev[p_][:],
                                    op=AOP.mult)
            d_t = gp.tile([128, H], F32, tag=f"d{p_}", name=f"d{p_}_{r}")
            nc.vector.tensor_tensor(d_t[:], u_t[:], sig[:, 0:H],
                                    op=AOP.subtract)
            v_t = gp.tile([128, H], F32, tag=f"v{p_}", name=f"v{p_}_{r}")
            nc.vector.tensor_tensor(v_t[:], sig[:, 0:H], sig[:, 3 * H:4 * H],
                                    op=AOP.mult)
            c_new = cp.tile([128, H], F32, tag=f"c{p_}", name=f"cn{p_}_{r}")
            nc.vector.scalar_tensor_tensor(c_new[:], v_t[:], 2.0, d_t[:],
                                           op0=AOP.mult, op1=AOP.add)
            tc_t = gp.tile([128, H], F32, tag=f"tc{p_}", name=f"tc{p_}_{r}")
            nc.scalar.activation(tc_t[:], c_new[:], AF.Tanh)
            c_prev[p_] = c_new
            st[p_]["tc"] = tc_t

        def s3(p_, r):
            u0 = 128 * p_
            hh = gp.tile([128, H], F16, tag=f"h{p_}", name=f"h{p_}_{r}")
            nc.vector.tensor_tensor(hh[:], st[p_]["sig"][:, 2 * H:3 * H],
                                    st[p_]["tc"][:], op=AOP.mult)
            pt = pst.tile([128, 128], F16, tag="pt", name=f"pt{p_}_{r}")
            nc.tensor.transpose(pt[0:H, :], hh[:], IDN[:])
            nc.vector.tensor_copy(zru[0:H, u0:u0 + 128, r + 1], pt[0:H, :])

        # software-pipelined: pipes staggered by a third of a round so each
        # engine alternates between the three dependency chains
        for p_ in range(NP):
            s1(p_, 0)
        for r in range(ROUNDS):
            for p_ in range(NP):
                s2(p_, r)
                s3(p_, r)
                if r + 1 < ROUNDS:
                    s1(p_, r + 1)

        ctxB.close()

        # ---- phase C: output layer ----------------------------------------
        ctxC = ExitStack()
        psc = ctxC.enter_context(tc.tile_pool(name="psc", bufs=3, space="PSUM"))
        zrv = ZR[:].rearrange("p (b j q) -> p b j q", b=BC, j=G, q=ZBLK)
        HC = big.tile([104, BC * TP], F16)

        # gather the valid h windows into time-order (per batch row) so the
        # output matmuls get 2D weight access patterns
        for b in range(BC):
            s4 = zrv[0:104, b, 0, W + 1:]
            src = bass.AP(s4.tensor, s4.offset,
                          [s4.ap[0], [ZBLK, G], [1, SEG]])
            eng = nc.sync if b % 2 == 0 else nc.scalar
            eng.dma_start(HC[:, TP * b:TP * (b + 1)], src)

        CH = [128, 128, 128, T - 3 * 128]   # t chunks per batch row
        for b in range(BC):
            for jg in range(4):
                w_ = CH[jg]
                zp = psc.tile([128, S], F32, tag="zp", name=f"zp{b}_{jg}")
                nc.tensor.matmul(zp[0:w_, :],
                                 HC[:, TP * b + 128 * jg:
                                    TP * b + 128 * jg + w_],
                                 WOB[:], start=True, stop=True,
                                 skip_group_check=True)
                sl = clp.tile([128, S], F16, tag="sl", name=f"sl{b}_{jg}")
                nc.scalar.activation(sl[0:w_, :], zp[0:w_, :], AF.Sigmoid)
                scr = clp.tile([128, S], F16, tag="scr", name=f"scr{b}_{jg}")
                eng = nc.vector if (b + jg) % 2 == 0 else nc.gpsimd
                eng.scalar_tensor_tensor(
                    scr[0:w_, :], sl[0:w_, :], 1.0,
                    QT[0:w_, 4 * S * b + S * jg:4 * S * b + S * (jg + 1)],
                    op0=AOP.mult, op1=AOP.mult,
                    accum_out=Y[0:w_, 4 * b + jg:4 * b + jg + 1])

        nc.sync.dma_start(yout_d.ap()[:], Y[:])
        ctxC.close()


# ---- host side -------------------------------------------------------------
def _prep(inputs):
    x = np.asarray(inputs["x"], np.float32)
    q = np.asarray(inputs["q"], np.float32)
    Wx = np.asarray(inputs["Wx"], np.float32)
    bx = np.asarray(inputs["bx"], np.float32)
    lstm_k = np.asarray(inputs["lstm_k"], np.float32)
    lstm_rk = np.asarray(inputs["lstm_rk"], np.float32)
    lstm_b = np.asarray(inputs["lstm_b"], np.float32)
    Wo = np.asarray(inputs["Wo"], np.float32)
    bo = np.asarray(inputs["bo"], np.float32)

    # channel de-interleave: deint[..., skill + 200*bit] = orig[..., 2*skill+bit]
    perm = np.empty(2 * S, np.int64)
    sk = np.arange(S)
    perm[sk] = 2 * sk
    perm[S + sk] = 2 * sk + 1

    xd = x[:, :, perm].transpose(0, 2, 1)                 # [B, 400, T]
    xdp = np.zeros((B, 4, E, TP), np.float16)
    xdp[:, :, :, :T] = xd.reshape(B, 4, E, T).astype(np.float16)
    xdp = xdp.transpose(0, 2, 1, 3).reshape(B, E, 4 * TP)

    # q blocked: part p = t % 128, col = (t // 128)*S + s
    qtmp = np.zeros((B, 4 * 128, S), np.float16)
    qtmp[:, :T, :] = q.astype(np.float16)
    qhp = np.ascontiguousarray(
        qtmp.reshape(B, 4, 128, S).transpose(0, 2, 1, 3).reshape(
            B, 128, 4 * S))

    # gate reorder [i,f,g,o] -> [i,f,o,g], then pre-scale the g section by 2
    # so the device computes tanh(g) = 2*sigmoid(2g) - 1 inside one sigmoid
    gperm = np.concatenate([np.arange(H), H + np.arange(H),
                            3 * H + np.arange(H), 2 * H + np.arange(H)])
    k_r = lstm_k[:, gperm].copy()
    rk_r = lstm_rk[:, gperm].copy()
    b_r = lstm_b[gperm].copy()
    k_r[:, 3 * H:] *= 2.0
    rk_r[:, 3 * H:] *= 2.0
    b_r[3 * H:] *= 2.0
    Wxd = Wx[perm]

    bias_row = bx @ k_r[:E] + b_r

    wxe = np.zeros((4, E, 128), np.float16)
    wxe[:, :, :E] = Wxd.reshape(4, E, E).astype(np.float16)

    kea = k_r[:E].astype(np.float16)                      # [E, 400]
    rka = np.zeros((104, 4 * H), np.float16)
    rka[0:H] = rk_r.astype(np.float16)
    rka[100:103] = k_r[E:E + 3].astype(np.float16)
    rka[103] = bias_row.astype(np.float16)

    wob = np.zeros((104, S), np.float16)
    wob[0:H] = Wo.astype(np.float16)
    wob[103] = bo.astype(np.float16)

    onesrow = np.zeros((1, NZC), np.float16)
    qq = np.arange(ZBLK)
    for j in range(G):
        tau = SEG * j - W + qq
        valid = (tau >= 0) & (tau < T)
        for b in range(BC):
            base = (b * G + j) * ZBLK
            onesrow[0, base:base + ZBLK][valid] = 1.0

    ident = np.eye(128, dtype=np.float16)
    return xdp, qhp, wxe, kea, rka, wob, onesrow, ident


def kernel(**inputs):
    if "nc" not in _cache:
        _cache["nc"] = _build()
    nc = _cache["nc"]

    xdp, qhp, wxe, kea, rka, wob, onesrow, ident = _prep(inputs)

    in_maps = []
    for c in range(NCORES):
        sl = slice(c * BC, (c + 1) * BC)
        in_maps.append({
            "xd": np.ascontiguousarray(xdp[sl]),
            "qh": np.ascontiguousarray(qhp[sl]),
            "wxe": wxe, "ke": kea, "rk": rka, "wob": wob,
            "onesrow": onesrow, "ident": ident,
        })

    res = run_bass_kernel_spmd(nc, in_maps, core_ids=list(range(NCORES)))

    y = np.zeros((B, T, 1), np.float32)
    for c in range(NCORES):
        yo = np.asarray(res.results[c]["yout"])     # [128, BC*4]
        yc = yo.reshape(128, BC, 4).transpose(1, 2, 0).reshape(BC, 4 * 128)
        y[c * BC:(c + 1) * BC, :, 0] = yc[:, :T]
    return y
